# revision 5
# baseline (speedup 1.0000x reference)
"""Trainium2 Bass kernel for nn_MinkConvBNRelu (sparse 3^3 conv + BN + ReLU).

Formulation: the scatter-add sparse conv is inverted on the host into a pure
gather form -- out[n] = sum_k feats[inv_idx[k, n]] @ W[k] -- by inverting the
per-offset (in_idx, out_idx, mask) pair lists (out_idx is unique within each
offset). The host then unfolds the gather (im2col-style) into a streamed
operand laid out exactly as the device GEMM consumes it: 7 groups of 4 offsets
stacked on the contraction dim (27 offsets padded to 28 with a zero weight),
channel-major [ (kk,c), vox ] tiles of 512 voxels.

Device work per core (1/8 of the voxels, SPMD on 8 NeuronCores):
  - stream G tiles [128, 512] from HBM, 7 matmuls (float32r) accumulate the
    [32, 512] transposed output tile in PSUM
  - ScalarE evacuates PSUM -> SBUF while accumulating per-channel sum and
    sum-of-squares (BatchNorm batch statistics)
  - AllReduce [32, 2] statistics across the 8 cores
  - ScalarE applies y = relu(x * scale + shift) with the BN affine collapsed
    into per-channel scale/shift, VectorE transposes 32x32 blocks back to
    voxel-major, DMA writes the [15360, 32] shard
"""

import sys

sys.path.insert(0, "/opt/trn_rl_repo")

import numpy as np

import concourse.bacc as bacc
import concourse.bass as bass
import concourse.tile as tile
from concourse import mybir
from concourse.bass_utils import run_bass_kernel_spmd

# Problem constants (hardcoded per harness contract).
N_VOX = 120000
C = 32
KVOL = 27
BN_EPS = 1e-5
N_CORES = 8
VOX_PER_CORE = N_VOX // N_CORES          # 15000
TILE = 512
NT = (VOX_PER_CORE + TILE - 1) // TILE   # 30
VOX_PAD = NT * TILE                      # 15360
NG = 7                                   # offset groups of 4 (27 -> pad 28)
ZERO_ROW = N_VOX                         # index of the appended all-zero row

_compiled = None  # (nc, core_ids) cache


def _build_device_kernel():
    nc = bacc.Bacc()
    gstream = nc.declare_dram_parameter(
        "gstream", [NG, NT, 128, TILE], mybir.dt.float32r, isOutput=False)
    wstack = nc.declare_dram_parameter(
        "wstack", [NG, 128, C], mybir.dt.float32r, isOutput=False)
    gb = nc.declare_dram_parameter("gb", [C, 2], mybir.dt.float32, isOutput=False)
    y_out = nc.declare_dram_parameter(
        "y", [VOX_PAD, C], mybir.dt.float32, isOutput=True)

    cc_in = nc.dram_tensor("cc_in", [C, 2], mybir.dt.float32)
    cc_out = nc.dram_tensor("cc_out", [C, 2], mybir.dt.float32, addr_space="Shared")
    core_ids = list(range(N_CORES))

    f32r = mybir.dt.float32r
    ACT = mybir.ActivationFunctionType

    with tile.TileContext(nc) as tc:
        with (
            tc.tile_pool(name="const", bufs=1) as constp,
            tc.tile_pool(name="rhs", bufs=16) as rhsp,
            tc.tile_pool(name="psum", bufs=4, space="PSUM") as psump,
            tc.tile_pool(name="ybuf", bufs=1) as ybufp,
            tc.tile_pool(name="small", bufs=1) as smallp,
            tc.tile_pool(name="outs", bufs=4) as outp,
        ):
            # Constants: weight stack [128, 7*32], gamma/beta [32, 2].
            wst = constp.tile([128, NG * C], mybir.dt.float32r)
            for g in range(NG):
                nc.sync.dma_start(out=wst[:, g * C:(g + 1) * C], in_=wstack[g])
            gb_t = constp.tile([C, 2], mybir.dt.float32)
            nc.sync.dma_start(out=gb_t[:], in_=gb[:])

            # Transposed activations accumulate here: Y^T [32, 15360].
            Y = ybufp.tile([C, VOX_PAD], mybir.dt.float32)
            sq_scratch = smallp.tile([C, TILE], mybir.dt.float32)
            sumx = smallp.tile([C, NT], mybir.dt.float32)
            sumsq = smallp.tile([C, NT], mybir.dt.float32)

            # Main loop: stream G tiles, matmul-accumulate, evac + stats.
            for t in range(NT):
                rhs = []
                for g in range(NG):
                    r = rhsp.tile([128, TILE], mybir.dt.float32r, tag="rhs")
                    nc.sync.dma_start(out=r[:], in_=gstream[g, t])
                    rhs.append(r)
                ps = psump.tile([C, TILE], mybir.dt.float32)
                for g in range(NG):
                    nc.tensor.matmul(
                        out=ps[:],
                        lhsT=wst[:, g * C:(g + 1) * C],
                        rhs=rhs[g][:],
                        start=(g == 0),
                        stop=(g == NG - 1),
                    )
                nc.scalar.activation(
                    out=Y[:, t * TILE:(t + 1) * TILE], in_=ps[:],
                    func=ACT.Identity, accum_out=sumx[:, t:t + 1])
                nc.scalar.activation(
                    out=sq_scratch[:], in_=ps[:],
                    func=ACT.Square, accum_out=sumsq[:, t:t + 1])

            # Reduce per-tile partial sums -> [32, 1] each, pack [32, 2].
            cc_sb = smallp.tile([C, 2], mybir.dt.float32)
            red_scratch = smallp.tile([C, NT], mybir.dt.float32)
            nc.scalar.activation(out=red_scratch[:], in_=sumx[:],
                                 func=ACT.Identity, accum_out=cc_sb[:, 0:1])
            nc.scalar.activation(out=red_scratch[:], in_=sumsq[:],
                                 func=ACT.Identity, accum_out=cc_sb[:, 1:2])

            nc.sync.dma_start(out=cc_in[:], in_=cc_sb[:])
            nc.gpsimd.collective_compute(
                "AllReduce", mybir.AluOpType.add,
                replica_groups=[core_ids],
                ins=[cc_in[:]], outs=[cc_out[:]],
            )
            cc2 = smallp.tile([C, 2], mybir.dt.float32)
            nc.sync.dma_start(out=cc2[:], in_=cc_out[:])

            # BN affine: scale = gamma * rsqrt(var + eps), shift = beta - mean*scale.
            mean = smallp.tile([C, 1], mybir.dt.float32)
            ex2 = smallp.tile([C, 1], mybir.dt.float32)
            msq = smallp.tile([C, 1], mybir.dt.float32)
            var = smallp.tile([C, 1], mybir.dt.float32)
            rstd = smallp.tile([C, 1], mybir.dt.float32)
            scale_v = smallp.tile([C, 1], mybir.dt.float32)
            tmp = smallp.tile([C, 1], mybir.dt.float32)
            shift_v = smallp.tile([C, 1], mybir.dt.float32)
            inv_n = 1.0 / float(N_VOX)
            nc.scalar.activation(out=mean[:], in_=cc2[:, 0:1], func=ACT.Copy, scale=inv_n)
            nc.scalar.activation(out=ex2[:], in_=cc2[:, 1:2], func=ACT.Copy, scale=inv_n)
            nc.scalar.activation(out=msq[:], in_=mean[:], func=ACT.Square)
            nc.vector.tensor_sub(out=var[:], in0=ex2[:], in1=msq[:])
            std = smallp.tile([C, 1], mybir.dt.float32)
            eps_t = smallp.tile([C, 1], mybir.dt.float32)
            nc.vector.memset(eps_t[:], BN_EPS)
            nc.vector.tensor_add(out=var[:], in0=var[:], in1=eps_t[:])
            nc.scalar.activation(out=std[:], in_=var[:], func=ACT.Sqrt)
            nc.vector.reciprocal(out=rstd[:], in_=std[:])
            nc.vector.tensor_mul(out=scale_v[:], in0=rstd[:], in1=gb_t[:, 0:1])
            nc.vector.tensor_mul(out=tmp[:], in0=mean[:], in1=scale_v[:])
            nc.vector.tensor_sub(out=shift_v[:], in0=gb_t[:, 1:2], in1=tmp[:])

            # Normalize + ReLU + transpose back to voxel-major + store.
            # y rows v = 512 t + 32 b + p  <- tt[p, 32 b + c]
            y_view = y_out[:].rearrange("(t b p) c -> t p b c", t=NT, b=TILE // 32, p=32)
            for t in range(NT):
                yr = outp.tile([C, TILE], mybir.dt.float32, tag="yr")
                nc.scalar.activation(
                    out=yr[:], in_=Y[:, t * TILE:(t + 1) * TILE],
                    func=ACT.Relu, bias=shift_v[:], scale=scale_v[:])
                tt = outp.tile([C, TILE], mybir.dt.float32, tag="tt")
                nc.vector.transpose(out=tt[:], in_=yr[:])
                nc.sync.dma_start(out=y_view[t], in_=tt[:])

    nc.compile()
    return nc, core_ids


def _prepare_inputs(feats, W, gamma, beta, in_idx, out_idx, mask):
    feats = np.ascontiguousarray(np.asarray(feats, np.float32))
    W = np.asarray(W, np.float32)
    in_idx = np.asarray(in_idx, np.int64)
    out_idx = np.asarray(out_idx, np.int64)
    mask = np.asarray(mask, bool)

    # Invert the per-offset pair lists: INV[k, n] = in-row feeding output n.
    INV = np.full((KVOL + 1, N_VOX), ZERO_ROW, np.int64)
    for k in range(KVOL):
        m = mask[k]
        INV[k, out_idx[k, m]] = in_idx[k, m]

    F1 = np.concatenate([feats, np.zeros((1, C), np.float32)], axis=0)

    # Weight stack [7, 128, 32] (pad offset 27 with zeros).
    W28 = np.concatenate([W, np.zeros((1, C, C), np.float32)], axis=0)
    wstack = np.ascontiguousarray(W28.reshape(NG, 4 * C, C), np.float32)
    gb = np.ascontiguousarray(np.stack(
        [np.asarray(gamma, np.float32), np.asarray(beta, np.float32)], axis=1))

    in_maps = []
    for r in range(N_CORES):
        idx_pad = np.full((KVOL + 1, VOX_PAD), ZERO_ROW, np.int64)
        idx_pad[:, :VOX_PER_CORE] = INV[:, r * VOX_PER_CORE:(r + 1) * VOX_PER_CORE]
        gs = np.empty((NG, NT, 128, TILE), np.float32)
        for g in range(NG):
            for kk in range(4):
                rows = F1[idx_pad[4 * g + kk]]                    # [15360, 32]
                gs[g, :, kk * C:(kk + 1) * C, :] = (
                    rows.reshape(NT, TILE, C).transpose(0, 2, 1))
            # offset 27 (g=6, kk=3) contributes zeros via idx_pad -> F1 zero row
        in_maps.append({"gstream": gs, "wstack": wstack, "gb": gb})
    return in_maps


def kernel(feats, W, gamma, beta, in_idx, out_idx, mask):
    global _compiled
    if _compiled is None:
        _compiled = _build_device_kernel()
    nc, core_ids = _compiled

    in_maps = _prepare_inputs(feats, W, gamma, beta, in_idx, out_idx, mask)
    res = run_bass_kernel_spmd(nc, in_maps, core_ids)

    out = np.empty((N_VOX, C), np.float32)
    for r in range(N_CORES):
        out[r * VOX_PER_CORE:(r + 1) * VOX_PER_CORE] = (
            res.results[r]["y"][:VOX_PER_CORE])
    return out


# revision 6
# speedup vs baseline: 1.0324x; 1.0324x over previous
"""Trainium2 Bass kernel for nn_MinkConvBNRelu (sparse 3^3 conv + BN + ReLU).

Formulation: the scatter-add sparse conv is inverted on the host into a pure
gather form -- out[n] = sum_k feats[inv_idx[k, n]] @ W[k] -- by inverting the
per-offset (in_idx, out_idx, mask) pair lists (out_idx is unique within each
offset). The host then unfolds the gather (im2col-style) into a streamed
operand laid out exactly as the device GEMM consumes it: 7 groups of 4 offsets
stacked on the contraction dim (27 offsets padded to 28 with a zero weight),
channel-major [ (kk,c), vox ] tiles of 512 voxels.

Device work per core (1/8 of the voxels, SPMD on 8 NeuronCores):
  - stream G tiles [128, 512] from HBM, 7 matmuls (float32r) accumulate the
    [32, 512] transposed output tile in PSUM
  - ScalarE evacuates PSUM -> SBUF while accumulating per-channel sum and
    sum-of-squares (BatchNorm batch statistics)
  - AllReduce [32, 2] statistics across the 8 cores
  - ScalarE applies y = relu(x * scale + shift) with the BN affine collapsed
    into per-channel scale/shift, VectorE transposes 32x32 blocks back to
    voxel-major, DMA writes the [15360, 32] shard
"""

import sys

sys.path.insert(0, "/opt/trn_rl_repo")

import numpy as np

import concourse.bacc as bacc
import concourse.bass as bass
import concourse.tile as tile
from concourse import mybir
from concourse.bass_utils import run_bass_kernel_spmd

# Problem constants (hardcoded per harness contract).
N_VOX = 120000
C = 32
KVOL = 27
BN_EPS = 1e-5
N_CORES = 8
VOX_PER_CORE = N_VOX // N_CORES          # 15000
TILE = 512
NT = (VOX_PER_CORE + TILE - 1) // TILE   # 30
VOX_PAD = NT * TILE                      # 15360
NG = 7                                   # offset groups of 4 (27 -> pad 28)
ZERO_ROW = N_VOX                         # index of the appended all-zero row

_compiled = None  # (nc, core_ids) cache


def _build_device_kernel():
    nc = bacc.Bacc()
    gstream = nc.declare_dram_parameter(
        "gstream", [NT, 128, NG * TILE], mybir.dt.float32r, isOutput=False)
    wstack = nc.declare_dram_parameter(
        "wstack", [NG, 128, C], mybir.dt.float32r, isOutput=False)
    gb = nc.declare_dram_parameter("gb", [C, 2], mybir.dt.float32, isOutput=False)
    y_out = nc.declare_dram_parameter(
        "y", [C, VOX_PAD], mybir.dt.float32, isOutput=True)

    cc_in = nc.dram_tensor("cc_in", [C, 2], mybir.dt.float32)
    cc_out = nc.dram_tensor("cc_out", [C, 2], mybir.dt.float32, addr_space="Shared")
    core_ids = list(range(N_CORES))

    f32r = mybir.dt.float32r
    ACT = mybir.ActivationFunctionType

    with tile.TileContext(nc) as tc:
        with (
            tc.tile_pool(name="const", bufs=1) as constp,
            tc.tile_pool(name="rhs", bufs=4) as rhsp,
            tc.tile_pool(name="psum", bufs=4, space="PSUM") as psump,
            tc.tile_pool(name="ybuf", bufs=1) as ybufp,
            tc.tile_pool(name="small", bufs=1) as smallp,
            tc.tile_pool(name="outs", bufs=4) as outp,
        ):
            # Constants: weight stack [128, 7*32], gamma/beta [32, 2].
            wst = constp.tile([128, NG * C], mybir.dt.float32r)
            for g in range(NG):
                nc.sync.dma_start(out=wst[:, g * C:(g + 1) * C], in_=wstack[g])
            gb_t = constp.tile([C, 2], mybir.dt.float32)
            nc.sync.dma_start(out=gb_t[:], in_=gb[:])

            # Transposed activations accumulate here: Y^T [32, 15360].
            Y = ybufp.tile([C, VOX_PAD], mybir.dt.float32)
            sq_scratch = smallp.tile([C, TILE], mybir.dt.float32)
            sumx = smallp.tile([C, NT], mybir.dt.float32)
            sumsq = smallp.tile([C, NT], mybir.dt.float32)

            # Main loop: stream G tiles, matmul-accumulate, evac + stats.
            for t in range(NT):
                rhs_t = rhsp.tile([128, NG * TILE], mybir.dt.float32r, tag="rhs")
                nc.sync.dma_start(out=rhs_t[:], in_=gstream[t])
                ps = psump.tile([C, TILE], mybir.dt.float32)
                for g in range(NG):
                    nc.tensor.matmul(
                        out=ps[:],
                        lhsT=wst[:, g * C:(g + 1) * C],
                        rhs=rhs_t[:, g * TILE:(g + 1) * TILE],
                        start=(g == 0),
                        stop=(g == NG - 1),
                    )
                nc.scalar.activation(
                    out=Y[:, t * TILE:(t + 1) * TILE], in_=ps[:],
                    func=ACT.Identity, accum_out=sumx[:, t:t + 1])
                nc.scalar.activation(
                    out=sq_scratch[:], in_=ps[:],
                    func=ACT.Square, accum_out=sumsq[:, t:t + 1])

            # Reduce per-tile partial sums -> [32, 1] each, pack [32, 2].
            cc_sb = smallp.tile([C, 2], mybir.dt.float32)
            red_scratch = smallp.tile([C, NT], mybir.dt.float32)
            nc.scalar.activation(out=red_scratch[:], in_=sumx[:],
                                 func=ACT.Identity, accum_out=cc_sb[:, 0:1])
            nc.scalar.activation(out=red_scratch[:], in_=sumsq[:],
                                 func=ACT.Identity, accum_out=cc_sb[:, 1:2])

            nc.sync.dma_start(out=cc_in[:], in_=cc_sb[:])
            nc.gpsimd.collective_compute(
                "AllReduce", mybir.AluOpType.add,
                replica_groups=[core_ids],
                ins=[cc_in[:]], outs=[cc_out[:]],
            )
            cc2 = smallp.tile([C, 2], mybir.dt.float32)
            nc.sync.dma_start(out=cc2[:], in_=cc_out[:])

            # BN affine: scale = gamma * rsqrt(var + eps), shift = beta - mean*scale.
            mean = smallp.tile([C, 1], mybir.dt.float32)
            ex2 = smallp.tile([C, 1], mybir.dt.float32)
            msq = smallp.tile([C, 1], mybir.dt.float32)
            var = smallp.tile([C, 1], mybir.dt.float32)
            rstd = smallp.tile([C, 1], mybir.dt.float32)
            scale_v = smallp.tile([C, 1], mybir.dt.float32)
            tmp = smallp.tile([C, 1], mybir.dt.float32)
            shift_v = smallp.tile([C, 1], mybir.dt.float32)
            inv_n = 1.0 / float(N_VOX)
            nc.scalar.activation(out=mean[:], in_=cc2[:, 0:1], func=ACT.Copy, scale=inv_n)
            nc.scalar.activation(out=ex2[:], in_=cc2[:, 1:2], func=ACT.Copy, scale=inv_n)
            nc.scalar.activation(out=msq[:], in_=mean[:], func=ACT.Square)
            nc.vector.tensor_sub(out=var[:], in0=ex2[:], in1=msq[:])
            std = smallp.tile([C, 1], mybir.dt.float32)
            eps_t = smallp.tile([C, 1], mybir.dt.float32)
            nc.vector.memset(eps_t[:], BN_EPS)
            nc.vector.tensor_add(out=var[:], in0=var[:], in1=eps_t[:])
            nc.scalar.activation(out=std[:], in_=var[:], func=ACT.Sqrt)
            nc.vector.reciprocal(out=rstd[:], in_=std[:])
            nc.vector.tensor_mul(out=scale_v[:], in0=rstd[:], in1=gb_t[:, 0:1])
            nc.vector.tensor_mul(out=tmp[:], in0=mean[:], in1=scale_v[:])
            nc.vector.tensor_sub(out=shift_v[:], in0=gb_t[:, 1:2], in1=tmp[:])

            # Normalize + ReLU; store channel-major (host transposes at the end).
            for t in range(NT):
                yr = outp.tile([C, TILE], mybir.dt.float32, tag="yr")
                nc.scalar.activation(
                    out=yr[:], in_=Y[:, t * TILE:(t + 1) * TILE],
                    func=ACT.Relu, bias=shift_v[:], scale=scale_v[:])
                nc.sync.dma_start(out=y_out[:, t * TILE:(t + 1) * TILE], in_=yr[:])

    nc.compile()
    return nc, core_ids


def _prepare_inputs(feats, W, gamma, beta, in_idx, out_idx, mask):
    feats = np.ascontiguousarray(np.asarray(feats, np.float32))
    W = np.asarray(W, np.float32)
    in_idx = np.asarray(in_idx, np.int64)
    out_idx = np.asarray(out_idx, np.int64)
    mask = np.asarray(mask, bool)

    # Invert the per-offset pair lists: INV[k, n] = in-row feeding output n.
    INV = np.full((KVOL + 1, N_VOX), ZERO_ROW, np.int64)
    for k in range(KVOL):
        m = mask[k]
        INV[k, out_idx[k, m]] = in_idx[k, m]

    F1 = np.concatenate([feats, np.zeros((1, C), np.float32)], axis=0)

    # Weight stack [7, 128, 32] (pad offset 27 with zeros).
    W28 = np.concatenate([W, np.zeros((1, C, C), np.float32)], axis=0)
    wstack = np.ascontiguousarray(W28.reshape(NG, 4 * C, C), np.float32)
    gb = np.ascontiguousarray(np.stack(
        [np.asarray(gamma, np.float32), np.asarray(beta, np.float32)], axis=1))

    in_maps = []
    for r in range(N_CORES):
        idx_pad = np.full((KVOL + 1, VOX_PAD), ZERO_ROW, np.int64)
        idx_pad[:, :VOX_PER_CORE] = INV[:, r * VOX_PER_CORE:(r + 1) * VOX_PER_CORE]
        gs = np.empty((NT, 128, NG, TILE), np.float32)
        for g in range(NG):
            for kk in range(4):
                rows = F1[idx_pad[4 * g + kk]]                    # [15360, 32]
                gs[:, kk * C:(kk + 1) * C, g, :] = (
                    rows.reshape(NT, TILE, C).transpose(0, 2, 1))
            # offset 27 (g=6, kk=3) contributes zeros via idx_pad -> F1 zero row
        gs = gs.reshape(NT, 128, NG * TILE)
        in_maps.append({"gstream": gs, "wstack": wstack, "gb": gb})
    return in_maps


def kernel(feats, W, gamma, beta, in_idx, out_idx, mask):
    global _compiled
    if _compiled is None:
        _compiled = _build_device_kernel()
    nc, core_ids = _compiled

    in_maps = _prepare_inputs(feats, W, gamma, beta, in_idx, out_idx, mask)
    res = run_bass_kernel_spmd(nc, in_maps, core_ids)

    out = np.empty((N_VOX, C), np.float32)
    for r in range(N_CORES):
        out[r * VOX_PER_CORE:(r + 1) * VOX_PER_CORE] = (
            res.results[r]["y"][:, :VOX_PER_CORE].T)
    return out


# revision 7
# speedup vs baseline: 1.0928x; 1.0586x over previous
"""Trainium2 Bass kernel for nn_MinkConvBNRelu (sparse 3^3 conv + BN + ReLU).

Formulation: the scatter-add sparse conv is inverted on the host into a pure
gather form -- out[n] = sum_k feats[inv_idx[k, n]] @ W[k] -- by inverting the
per-offset (in_idx, out_idx, mask) pair lists (out_idx is unique within each
offset). The host then unfolds the gather (im2col-style) into a streamed
operand laid out exactly as the device GEMM consumes it: 7 groups of 4 offsets
stacked on the contraction dim (27 offsets padded to 28 with a zero weight),
channel-major [ (kk,c), vox ] tiles of 512 voxels.

Device work per core (1/8 of the voxels, SPMD on 8 NeuronCores):
  - stream G tiles [128, 512] from HBM, 7 matmuls (float32r) accumulate the
    [32, 512] transposed output tile in PSUM
  - ScalarE evacuates PSUM -> SBUF while accumulating per-channel sum and
    sum-of-squares (BatchNorm batch statistics)
  - AllReduce [32, 2] statistics across the 8 cores
  - ScalarE applies y = relu(x * scale + shift) with the BN affine collapsed
    into per-channel scale/shift, VectorE transposes 32x32 blocks back to
    voxel-major, DMA writes the [15360, 32] shard
"""

import sys

sys.path.insert(0, "/opt/trn_rl_repo")

import numpy as np

import concourse.bacc as bacc
import concourse.bass as bass
import concourse.tile as tile
from concourse import mybir
from concourse.bass_utils import run_bass_kernel_spmd

# Problem constants (hardcoded per harness contract).
N_VOX = 120000
C = 32
KVOL = 27
BN_EPS = 1e-5
N_CORES = 8
VOX_PER_CORE = N_VOX // N_CORES          # 15000
TILE = 512
NT = (VOX_PER_CORE + TILE - 1) // TILE   # 30
VOX_PAD = NT * TILE                      # 15360
NG = 7                                   # offset groups of 4 (27 -> pad 28)
ZERO_ROW = N_VOX                         # index of the appended all-zero row

_compiled = None  # (nc, core_ids) cache


def _build_device_kernel():
    nc = bacc.Bacc()
    gstream = nc.declare_dram_parameter(
        "gstream", [NT, 128, NG * TILE], mybir.dt.float32r, isOutput=False)
    wstack = nc.declare_dram_parameter(
        "wstack", [NG, 128, C], mybir.dt.float32r, isOutput=False)
    gb = nc.declare_dram_parameter("gb", [C, 2], mybir.dt.float32, isOutput=False)
    y_out = nc.declare_dram_parameter(
        "y", [C, VOX_PAD], mybir.dt.float32, isOutput=True)

    cc_in = nc.dram_tensor("cc_in", [C, 2], mybir.dt.float32)
    cc_out = nc.dram_tensor("cc_out", [C, 2], mybir.dt.float32, addr_space="Shared")
    core_ids = list(range(N_CORES))

    f32r = mybir.dt.float32r
    ACT = mybir.ActivationFunctionType

    with tile.TileContext(nc) as tc:
        with (
            tc.tile_pool(name="const", bufs=1) as constp,
            tc.tile_pool(name="rhs", bufs=6) as rhsp,
            tc.tile_pool(name="psum", bufs=4, space="PSUM") as psump,
            tc.tile_pool(name="ybuf", bufs=1) as ybufp,
            tc.tile_pool(name="small", bufs=1) as smallp,
            tc.tile_pool(name="outs", bufs=1) as outp,
        ):
            # Constants: weight stack [128, 7*32], gamma/beta [32, 2].
            wst = constp.tile([128, NG * C], mybir.dt.float32r)
            for g in range(NG):
                nc.sync.dma_start(out=wst[:, g * C:(g + 1) * C], in_=wstack[g])
            gb_t = constp.tile([C, 2], mybir.dt.float32)
            nc.sync.dma_start(out=gb_t[:], in_=gb[:])

            # Transposed activations accumulate here: Y^T [32, 15360].
            Y = ybufp.tile([C, VOX_PAD], mybir.dt.float32)
            sq_scratch = smallp.tile([C, TILE], mybir.dt.float32)
            sumx = smallp.tile([C, NT], mybir.dt.float32)
            sumsq = smallp.tile([C, NT], mybir.dt.float32)

            # Main loop: stream G tiles, matmul-accumulate, evac + stats.
            for t in range(NT):
                rhs_t = rhsp.tile([128, NG * TILE], mybir.dt.float32r, tag="rhs")
                nc.sync.dma_start(out=rhs_t[:], in_=gstream[t])
                ps = psump.tile([C, TILE], mybir.dt.float32)
                for g in range(NG):
                    nc.tensor.matmul(
                        out=ps[:],
                        lhsT=wst[:, g * C:(g + 1) * C],
                        rhs=rhs_t[:, g * TILE:(g + 1) * TILE],
                        start=(g == 0),
                        stop=(g == NG - 1),
                    )
                nc.scalar.activation(
                    out=Y[:, t * TILE:(t + 1) * TILE], in_=ps[:],
                    func=ACT.Identity, accum_out=sumx[:, t:t + 1])
                nc.scalar.activation(
                    out=sq_scratch[:], in_=ps[:],
                    func=ACT.Square, accum_out=sumsq[:, t:t + 1])

            # Reduce per-tile partial sums -> [32, 1] each, pack [32, 2].
            cc_sb = smallp.tile([C, 2], mybir.dt.float32)
            red_scratch = smallp.tile([C, NT], mybir.dt.float32)
            nc.scalar.activation(out=red_scratch[:], in_=sumx[:],
                                 func=ACT.Identity, accum_out=cc_sb[:, 0:1])
            nc.scalar.activation(out=red_scratch[:], in_=sumsq[:],
                                 func=ACT.Identity, accum_out=cc_sb[:, 1:2])

            nc.sync.dma_start(out=cc_in[:], in_=cc_sb[:])
            nc.gpsimd.collective_compute(
                "AllReduce", mybir.AluOpType.add,
                replica_groups=[core_ids],
                ins=[cc_in[:]], outs=[cc_out[:]],
            )
            cc2 = smallp.tile([C, 2], mybir.dt.float32)
            nc.sync.dma_start(out=cc2[:], in_=cc_out[:])

            # BN affine: scale = gamma * rsqrt(var + eps), shift = beta - mean*scale.
            mean = smallp.tile([C, 1], mybir.dt.float32)
            ex2 = smallp.tile([C, 1], mybir.dt.float32)
            msq = smallp.tile([C, 1], mybir.dt.float32)
            var = smallp.tile([C, 1], mybir.dt.float32)
            rstd = smallp.tile([C, 1], mybir.dt.float32)
            scale_v = smallp.tile([C, 1], mybir.dt.float32)
            tmp = smallp.tile([C, 1], mybir.dt.float32)
            shift_v = smallp.tile([C, 1], mybir.dt.float32)
            inv_n = 1.0 / float(N_VOX)
            nc.scalar.activation(out=mean[:], in_=cc2[:, 0:1], func=ACT.Copy, scale=inv_n)
            nc.scalar.activation(out=ex2[:], in_=cc2[:, 1:2], func=ACT.Copy, scale=inv_n)
            nc.scalar.activation(out=msq[:], in_=mean[:], func=ACT.Square)
            nc.vector.tensor_sub(out=var[:], in0=ex2[:], in1=msq[:])
            std = smallp.tile([C, 1], mybir.dt.float32)
            eps_t = smallp.tile([C, 1], mybir.dt.float32)
            nc.vector.memset(eps_t[:], BN_EPS)
            nc.vector.tensor_add(out=var[:], in0=var[:], in1=eps_t[:])
            nc.scalar.activation(out=std[:], in_=var[:], func=ACT.Sqrt)
            nc.vector.reciprocal(out=rstd[:], in_=std[:])
            nc.vector.tensor_mul(out=scale_v[:], in0=rstd[:], in1=gb_t[:, 0:1])
            nc.vector.tensor_mul(out=tmp[:], in0=mean[:], in1=scale_v[:])
            nc.vector.tensor_sub(out=shift_v[:], in0=gb_t[:, 1:2], in1=tmp[:])

            # Normalize + ReLU in one pass; store channel-major
            # (host transposes at the end).
            yr = outp.tile([C, VOX_PAD], mybir.dt.float32, tag="yr")
            nc.scalar.activation(
                out=yr[:], in_=Y[:],
                func=ACT.Relu, bias=shift_v[:], scale=scale_v[:])
            nc.sync.dma_start(out=y_out[:], in_=yr[:])

    nc.compile()
    return nc, core_ids


def _prepare_inputs(feats, W, gamma, beta, in_idx, out_idx, mask):
    feats = np.ascontiguousarray(np.asarray(feats, np.float32))
    W = np.asarray(W, np.float32)
    in_idx = np.asarray(in_idx, np.int64)
    out_idx = np.asarray(out_idx, np.int64)
    mask = np.asarray(mask, bool)

    # Invert the per-offset pair lists: INV[k, n] = in-row feeding output n.
    INV = np.full((KVOL + 1, N_VOX), ZERO_ROW, np.int64)
    for k in range(KVOL):
        m = mask[k]
        INV[k, out_idx[k, m]] = in_idx[k, m]

    F1 = np.concatenate([feats, np.zeros((1, C), np.float32)], axis=0)

    # Weight stack [7, 128, 32] (pad offset 27 with zeros).
    W28 = np.concatenate([W, np.zeros((1, C, C), np.float32)], axis=0)
    wstack = np.ascontiguousarray(W28.reshape(NG, 4 * C, C), np.float32)
    gb = np.ascontiguousarray(np.stack(
        [np.asarray(gamma, np.float32), np.asarray(beta, np.float32)], axis=1))

    in_maps = []
    for r in range(N_CORES):
        idx_pad = np.full((KVOL + 1, VOX_PAD), ZERO_ROW, np.int64)
        idx_pad[:, :VOX_PER_CORE] = INV[:, r * VOX_PER_CORE:(r + 1) * VOX_PER_CORE]
        gs = np.empty((NT, 128, NG, TILE), np.float32)
        for g in range(NG):
            for kk in range(4):
                rows = F1[idx_pad[4 * g + kk]]                    # [15360, 32]
                gs[:, kk * C:(kk + 1) * C, g, :] = (
                    rows.reshape(NT, TILE, C).transpose(0, 2, 1))
            # offset 27 (g=6, kk=3) contributes zeros via idx_pad -> F1 zero row
        gs = gs.reshape(NT, 128, NG * TILE)
        in_maps.append({"gstream": gs, "wstack": wstack, "gb": gb})
    return in_maps


def kernel(feats, W, gamma, beta, in_idx, out_idx, mask):
    global _compiled
    if _compiled is None:
        _compiled = _build_device_kernel()
    nc, core_ids = _compiled

    in_maps = _prepare_inputs(feats, W, gamma, beta, in_idx, out_idx, mask)
    res = run_bass_kernel_spmd(nc, in_maps, core_ids)

    out = np.empty((N_VOX, C), np.float32)
    for r in range(N_CORES):
        out[r * VOX_PER_CORE:(r + 1) * VOX_PER_CORE] = (
            res.results[r]["y"][:, :VOX_PER_CORE].T)
    return out


# revision 8
# speedup vs baseline: 1.1844x; 1.0837x over previous
"""Trainium2 Bass kernel for nn_MinkConvBNRelu (sparse 3^3 conv + BN + ReLU).

Formulation: the scatter-add sparse conv is inverted on the host into a pure
gather form -- out[n] = sum_k feats[inv_idx[k, n]] @ W[k] -- by inverting the
per-offset (in_idx, out_idx, mask) pair lists (out_idx is unique within each
offset). The host then unfolds the gather (im2col-style) into a streamed
operand laid out exactly as the device GEMM consumes it: 7 groups of 4 offsets
stacked on the contraction dim (27 offsets padded to 28 with a zero weight),
channel-major [ (kk,c), vox ] tiles of 512 voxels.

Device work per core (1/8 of the voxels, SPMD on 8 NeuronCores):
  - stream G tiles [128, 512] from HBM, 7 matmuls (float32r) accumulate the
    [32, 512] transposed output tile in PSUM
  - ScalarE evacuates PSUM -> SBUF while accumulating per-channel sum and
    sum-of-squares (BatchNorm batch statistics)
  - AllReduce [32, 2] statistics across the 8 cores
  - ScalarE applies y = relu(x * scale + shift) with the BN affine collapsed
    into per-channel scale/shift, VectorE transposes 32x32 blocks back to
    voxel-major, DMA writes the [15360, 32] shard
"""

import sys

sys.path.insert(0, "/opt/trn_rl_repo")

import numpy as np

import concourse.bacc as bacc
import concourse.bass as bass
import concourse.tile as tile
from concourse import mybir
from concourse.bass_utils import run_bass_kernel_spmd

# Problem constants (hardcoded per harness contract).
N_VOX = 120000
C = 32
KVOL = 27
BN_EPS = 1e-5
N_CORES = 8
VOX_PER_CORE = N_VOX // N_CORES          # 15000
TILE = 512
NT = (VOX_PER_CORE + TILE - 1) // TILE   # 30
VOX_PAD = NT * TILE                      # 15360
NG = 7                                   # offset groups of 4 (27 -> pad 28)
ZERO_ROW = N_VOX                         # index of the appended all-zero row

_compiled = None  # (nc, core_ids) cache


def _build_device_kernel():
    nc = bacc.Bacc()
    gstream = nc.declare_dram_parameter(
        "gstream", [NT, 128, NG * TILE], mybir.dt.float32r, isOutput=False)
    wstack = nc.declare_dram_parameter(
        "wstack", [NG, 128, C], mybir.dt.float32r, isOutput=False)
    gb = nc.declare_dram_parameter("gb", [C, 2], mybir.dt.float32, isOutput=False)
    y_out = nc.declare_dram_parameter(
        "y", [C, VOX_PAD], mybir.dt.float32, isOutput=True)

    cc_in = nc.dram_tensor("cc_in", [C, 2], mybir.dt.float32)
    cc_out = nc.dram_tensor("cc_out", [C, 2], mybir.dt.float32, addr_space="Shared")
    cc_warm_in = nc.dram_tensor("cc_warm_in", [C, 2], mybir.dt.float32)
    cc_warm_out = nc.dram_tensor("cc_warm_out", [C, 2], mybir.dt.float32, addr_space="Shared")
    core_ids = list(range(N_CORES))

    f32r = mybir.dt.float32r
    ACT = mybir.ActivationFunctionType

    with tile.TileContext(nc) as tc:
        with (
            tc.tile_pool(name="const", bufs=1) as constp,
            tc.tile_pool(name="rhs", bufs=6) as rhsp,
            tc.tile_pool(name="psum", bufs=4, space="PSUM") as psump,
            tc.tile_pool(name="ybuf", bufs=1) as ybufp,
            tc.tile_pool(name="small", bufs=1) as smallp,
            tc.tile_pool(name="outs", bufs=2) as outp,
        ):
            # Constants: weight stack [128, 7*32], gamma/beta [32, 2].
            wst = constp.tile([128, NG * C], mybir.dt.float32r)
            for g in range(NG):
                nc.sync.dma_start(out=wst[:, g * C:(g + 1) * C], in_=wstack[g])
            gb_t = constp.tile([C, 2], mybir.dt.float32)
            nc.sync.dma_start(out=gb_t[:], in_=gb[:])

            # Warm-ups, overlapped with the DMA-bound main loop: ncfw/TOPSP
            # collective context and the ACT tables for Sqrt/Relu.
            warm = smallp.tile([C, 2], mybir.dt.float32)
            nc.vector.memset(warm[:], 0.0)
            nc.sync.dma_start(out=cc_warm_in[:], in_=warm[:])
            nc.gpsimd.collective_compute(
                "AllReduce", mybir.AluOpType.add,
                replica_groups=[core_ids],
                ins=[cc_warm_in[:]], outs=[cc_warm_out[:]],
            )
            wsc = smallp.tile([C, 1], mybir.dt.float32)
            nc.scalar.activation(out=wsc[:], in_=gb_t[:, 0:1], func=ACT.Sqrt)
            nc.scalar.activation(out=wsc[:], in_=gb_t[:, 0:1], func=ACT.Relu)

            # Transposed activations accumulate here: Y^T [32, 15360].
            Y = ybufp.tile([C, VOX_PAD], mybir.dt.float32)
            sq_scratch = smallp.tile([C, TILE], mybir.dt.float32)
            sumx = smallp.tile([C, NT], mybir.dt.float32)
            sumsq = smallp.tile([C, NT], mybir.dt.float32)

            # Main loop: stream G tiles, matmul-accumulate, evac + stats.
            for t in range(NT):
                rhs_t = rhsp.tile([128, NG * TILE], mybir.dt.float32r, tag="rhs")
                nc.sync.dma_start(out=rhs_t[:], in_=gstream[t])
                ps = psump.tile([C, TILE], mybir.dt.float32)
                for g in range(NG):
                    nc.tensor.matmul(
                        out=ps[:],
                        lhsT=wst[:, g * C:(g + 1) * C],
                        rhs=rhs_t[:, g * TILE:(g + 1) * TILE],
                        start=(g == 0),
                        stop=(g == NG - 1),
                    )
                nc.scalar.activation(
                    out=Y[:, t * TILE:(t + 1) * TILE], in_=ps[:],
                    func=ACT.Identity, accum_out=sumx[:, t:t + 1])
                nc.scalar.activation(
                    out=sq_scratch[:], in_=ps[:],
                    func=ACT.Square, accum_out=sumsq[:, t:t + 1])

            # Reduce per-tile partial sums -> [32, 1] each, pack [32, 2].
            cc_sb = smallp.tile([C, 2], mybir.dt.float32)
            red_scratch = smallp.tile([C, NT], mybir.dt.float32)
            nc.scalar.activation(out=red_scratch[:], in_=sumx[:],
                                 func=ACT.Identity, accum_out=cc_sb[:, 0:1])
            nc.scalar.activation(out=red_scratch[:], in_=sumsq[:],
                                 func=ACT.Identity, accum_out=cc_sb[:, 1:2])

            nc.sync.dma_start(out=cc_in[:], in_=cc_sb[:])
            nc.gpsimd.collective_compute(
                "AllReduce", mybir.AluOpType.add,
                replica_groups=[core_ids],
                ins=[cc_in[:]], outs=[cc_out[:]],
            )
            cc2 = smallp.tile([C, 2], mybir.dt.float32)
            nc.sync.dma_start(out=cc2[:], in_=cc_out[:])

            # BN affine: scale = gamma * rsqrt(var + eps), shift = beta - mean*scale.
            mean = smallp.tile([C, 1], mybir.dt.float32)
            ex2 = smallp.tile([C, 1], mybir.dt.float32)
            msq = smallp.tile([C, 1], mybir.dt.float32)
            var = smallp.tile([C, 1], mybir.dt.float32)
            rstd = smallp.tile([C, 1], mybir.dt.float32)
            scale_v = smallp.tile([C, 1], mybir.dt.float32)
            tmp = smallp.tile([C, 1], mybir.dt.float32)
            shift_v = smallp.tile([C, 1], mybir.dt.float32)
            inv_n = 1.0 / float(N_VOX)
            nc.scalar.activation(out=mean[:], in_=cc2[:, 0:1], func=ACT.Copy, scale=inv_n)
            nc.scalar.activation(out=ex2[:], in_=cc2[:, 1:2], func=ACT.Copy, scale=inv_n)
            nc.scalar.activation(out=msq[:], in_=mean[:], func=ACT.Square)
            nc.vector.tensor_sub(out=var[:], in0=ex2[:], in1=msq[:])
            std = smallp.tile([C, 1], mybir.dt.float32)
            eps_t = smallp.tile([C, 1], mybir.dt.float32)
            nc.vector.memset(eps_t[:], BN_EPS)
            nc.vector.tensor_add(out=var[:], in0=var[:], in1=eps_t[:])
            nc.scalar.activation(out=std[:], in_=var[:], func=ACT.Sqrt)
            nc.vector.reciprocal(out=rstd[:], in_=std[:])
            nc.vector.tensor_mul(out=scale_v[:], in0=rstd[:], in1=gb_t[:, 0:1])
            nc.vector.tensor_mul(out=tmp[:], in0=mean[:], in1=scale_v[:])
            nc.vector.tensor_sub(out=shift_v[:], in0=gb_t[:, 1:2], in1=tmp[:])

            # Normalize + ReLU, chunked so the store overlaps the next chunk.
            NCH = 4
            CH = VOX_PAD // NCH
            for i in range(NCH):
                yr = outp.tile([C, CH], mybir.dt.float32, tag="yr")
                nc.scalar.activation(
                    out=yr[:], in_=Y[:, i * CH:(i + 1) * CH],
                    func=ACT.Relu, bias=shift_v[:], scale=scale_v[:])
                nc.sync.dma_start(out=y_out[:, i * CH:(i + 1) * CH], in_=yr[:])

    nc.compile()
    return nc, core_ids


def _prepare_inputs(feats, W, gamma, beta, in_idx, out_idx, mask):
    feats = np.ascontiguousarray(np.asarray(feats, np.float32))
    W = np.asarray(W, np.float32)
    in_idx = np.asarray(in_idx, np.int64)
    out_idx = np.asarray(out_idx, np.int64)
    mask = np.asarray(mask, bool)

    # Invert the per-offset pair lists: INV[k, n] = in-row feeding output n.
    INV = np.full((KVOL + 1, N_VOX), ZERO_ROW, np.int64)
    for k in range(KVOL):
        m = mask[k]
        INV[k, out_idx[k, m]] = in_idx[k, m]

    F1 = np.concatenate([feats, np.zeros((1, C), np.float32)], axis=0)

    # Weight stack [7, 128, 32] (pad offset 27 with zeros).
    W28 = np.concatenate([W, np.zeros((1, C, C), np.float32)], axis=0)
    wstack = np.ascontiguousarray(W28.reshape(NG, 4 * C, C), np.float32)
    gb = np.ascontiguousarray(np.stack(
        [np.asarray(gamma, np.float32), np.asarray(beta, np.float32)], axis=1))

    in_maps = []
    for r in range(N_CORES):
        idx_pad = np.full((KVOL + 1, VOX_PAD), ZERO_ROW, np.int64)
        idx_pad[:, :VOX_PER_CORE] = INV[:, r * VOX_PER_CORE:(r + 1) * VOX_PER_CORE]
        gs = np.empty((NT, 128, NG, TILE), np.float32)
        for g in range(NG):
            for kk in range(4):
                rows = F1[idx_pad[4 * g + kk]]                    # [15360, 32]
                gs[:, kk * C:(kk + 1) * C, g, :] = (
                    rows.reshape(NT, TILE, C).transpose(0, 2, 1))
            # offset 27 (g=6, kk=3) contributes zeros via idx_pad -> F1 zero row
        gs = gs.reshape(NT, 128, NG * TILE)
        in_maps.append({"gstream": gs, "wstack": wstack, "gb": gb})
    return in_maps


def kernel(feats, W, gamma, beta, in_idx, out_idx, mask):
    global _compiled
    if _compiled is None:
        _compiled = _build_device_kernel()
    nc, core_ids = _compiled

    in_maps = _prepare_inputs(feats, W, gamma, beta, in_idx, out_idx, mask)
    res = run_bass_kernel_spmd(nc, in_maps, core_ids)

    out = np.empty((N_VOX, C), np.float32)
    for r in range(N_CORES):
        out[r * VOX_PER_CORE:(r + 1) * VOX_PER_CORE] = (
            res.results[r]["y"][:, :VOX_PER_CORE].T)
    return out
